# revision 41
# baseline (speedup 1.0000x reference)
"""Trainium2 Bass kernel for a dense Clebsch-Gordan tensor product + per-irrep Linear.

Reference computation (e3nn-style):
  x1: [N, 1152] = 128x0e + 128x1o + 128x2e   (mul=128, l=0,1,2)
  x2: [N, 9]    = 1x0e + 1x1o + 1x2e         (spherical harmonics)
  y[n, (g, v*d3+k)] = sum_{paths p in g} sum_{u,i,j} w_g[slot_p*128+u, v]/sqrt(mul_g)
                       * cg_p[i,j,k] * x1_{l1}[n,u,i] * x2_{l2}[n,j]

Strategy (data-parallel over N across 8 cores; per core n=1024):
  - host precomputes one scaled weight matrix W~[u,v] per CG nonzero (244, fp16),
    plus fp16 relayouts x1T[u; i_glob, n] and x2rep[128; j_glob, n]
  - DVE builds outer-product columns OP_{ij}[u; n] = x1T[:, i, :] * x2rep[:, j, :]
  - one fp16 matmul per CG nonzero accumulates y_psum[(g,k)][v; n] += W~.T @ OP_{ij}
  - PSUM drains (Act/Pool) write fp16 staging; DMA emits y in [v; gk, n] layout;
    the host performs the final [v, gk, n] -> [n, (g,v,k)] untranspose + fp32 cast
"""

import sys
from math import factorial

import numpy as np

if "/opt/trn_rl_repo" not in sys.path:
    sys.path.insert(0, "/opt/trn_rl_repo")

MUL = 128
N_TOTAL = 8192
N_CORES = 8
NPC = N_TOTAL // N_CORES  # 1024 rows per core
IN1 = [(0, 1), (1, -1), (2, 1)]
IN2 = [(0, 1), (1, -1), (2, 1)]

# --------------------------------------------------------------- CG tables ---


def _f(n):
    return float(factorial(n))


def _su2_cg(j1, j2, j3):
    C = np.zeros((2 * j1 + 1, 2 * j2 + 1, 2 * j3 + 1))
    if not (abs(j1 - j2) <= j3 <= j1 + j2):
        return C
    pref0 = np.sqrt((2 * j3 + 1) * _f(j1 + j2 - j3) * _f(j1 - j2 + j3) * _f(-j1 + j2 + j3) / _f(j1 + j2 + j3 + 1))
    for m1 in range(-j1, j1 + 1):
        for m2 in range(-j2, j2 + 1):
            m3 = m1 + m2
            if abs(m3) > j3:
                continue
            pref = pref0 * np.sqrt(_f(j3 + m3) * _f(j3 - m3) * _f(j1 - m1) * _f(j1 + m1) * _f(j2 - m2) * _f(j2 + m2))
            s = 0.0
            for k in range(j1 + j2 - j3 + 1):
                a = [k, j1 + j2 - j3 - k, j1 - m1 - k, j2 + m2 - k, j3 - j2 + m1 + k, j3 - j1 - m2 + k]
                if min(a) < 0:
                    continue
                s += (-1.0) ** k / np.prod([_f(t) for t in a])
            C[j1 + m1, j2 + m2, j3 + m3] = pref * s
    return C


def _q(l):
    q = np.zeros((2 * l + 1, 2 * l + 1), dtype=np.complex128)
    for m in range(-l, 0):
        q[l + m, l + abs(m)] = 1 / np.sqrt(2)
        q[l + m, l - abs(m)] = -1j / np.sqrt(2)
    q[l, l] = 1.0
    for m in range(1, l + 1):
        q[l + m, l + abs(m)] = (-1) ** m / np.sqrt(2)
        q[l + m, l - abs(m)] = 1j * (-1) ** m / np.sqrt(2)
    return (-1j) ** l * q


def _real_cg(l1, l2, l3):
    C = _su2_cg(l1, l2, l3).astype(np.complex128)
    C = np.einsum("ij,kl,mn,ikn->jlm", _q(l1), _q(l2), np.conj(_q(l3).T), C)
    return np.real(C)


PATHS = []
for (l1, p1) in IN1:
    for (l2, p2) in IN2:
        for l3 in range(abs(l1 - l2), l1 + l2 + 1):
            PATHS.append((l1, p1, l2, p2, l3, p1 * p2))
CG = {(l1, l2, l3): _real_cg(l1, l2, l3).astype(np.float32) for (l1, _, l2, _, l3, _) in PATHS}
GROUPS = sorted({(l3, p3) for (_, _, _, _, l3, p3) in PATHS})


def _gname(l, p):
    return "w%d%s" % (l, "e" if p == 1 else "o")


L1_OFF = {0: 0, 1: 1, 2: 4}   # i_glob = L1_OFF[l1] + i
X1_OFF = {0: 0, 1: 128, 2: 512}  # x1 flat col offset of l1 block
L2_OFF = {0: 0, 1: 1, 2: 4}   # j_glob = L2_OFF[l2] + j

MULS = {g: 0 for g in GROUPS}
for (_, _, _, _, l3, p3) in PATHS:
    MULS[(l3, p3)] += MUL

GOFF = {}
_off = 0
for g in GROUPS:
    GOFF[g] = _off
    _off += MUL * (2 * g[0] + 1)
assert _off == 5120


def _build_pass_list():
    """gk_passes: per (g,k) in output order, list of (ij, path_idx, coef)."""
    gk_passes = []
    for g in GROUPS:
        d3 = 2 * g[0] + 1
        for k in range(d3):
            contribs = []
            for pi, (l1, p1, l2, p2, l3, p3) in enumerate(PATHS):
                if (l3, p3) != g:
                    continue
                C = CG[(l1, l2, l3)]
                for i in range(2 * l1 + 1):
                    for j in range(2 * l2 + 1):
                        c = float(C[i, j, k])
                        if abs(c) < 1e-8:
                            continue
                        contribs.append(((L1_OFF[l1] + i, L2_OFF[l2] + j), pi, c))
            assert contribs
            gk_passes.append((g, k, contribs))
    used = []
    seen = set()
    for (_, _, contribs) in gk_passes:
        for (ij, _, _) in contribs:
            if ij not in seen:
                seen.add(ij)
                used.append(ij)
    return gk_passes, used


GK_PASSES, USED_IJ = _build_pass_list()
GK_BASE = {}
_i = 0
for _g in GROUPS:
    GK_BASE[_g] = _i
    _i += 2 * _g[0] + 1
N_GK = _i  # 40

N_PASSES = sum(len(c) for (_, _, c) in GK_PASSES)  # 244


def _build_mm_plan(combo_level):
    """Group each (path,k) slot's CG terms into units of proportional
    coefficients; units built as SBUF combo tiles collapse to one matmul.

    combo_level: 0 = no combos; 1 = only units recurring across slots;
    2 = every multi-term unit.

    Returns (mm_list, combos) where
      mm_list: per (g,k) in output order, list of (operand, pi, coef);
               operand = ('op', ij) or ('combo', q)
      combos:  list over q of tuple((ij, ratio), ...) defining
               combo_q = sum_t ratio_t * OP_{ij_t}  (ratio_0 == 1)
    """
    from collections import Counter, defaultdict

    slot_units = []  # per (g,k): list of (pattern, c0, pi) per path-slot
    unit_count = Counter()
    for (g, k, contribs) in GK_PASSES:
        by_path = defaultdict(list)
        for (ij, pi, c) in contribs:
            by_path[pi].append((ij, c))
        units = []
        for pi in sorted(by_path):
            byc = defaultdict(list)
            for (ij, c) in by_path[pi]:
                byc[round(abs(c), 5)].append((ij, c))
            for a in sorted(byc):
                lst = sorted(byc[a])
                c0 = lst[0][1]
                pat = tuple((ij, round(c / c0, 5)) for ij, c in lst)
                units.append((pat, c0, pi))
                unit_count[pat] += 1
        slot_units.append(units)

    combo_idx = {}
    combos = []
    mm_list = []
    for units in slot_units:
        singles, combo_mms = [], []
        for (pat, c0, pi) in units:
            use_combo = len(pat) >= 2 and (
                combo_level >= 2 or (combo_level == 1 and unit_count[pat] >= 2))
            if use_combo:
                if pat not in combo_idx:
                    combo_idx[pat] = len(combos)
                    combos.append(pat)
                combo_mms.append((('combo', combo_idx[pat]), pi, c0))
            else:
                for (ij, r) in pat:
                    singles.append((('op', ij), pi, c0 * r))
        # singles first: PE can start them while DVE still builds combos
        mm_list.append(singles + combo_mms)
    return mm_list, combos


_PLAN_CACHE = {}


def _plan(combo_level):
    if combo_level not in _PLAN_CACHE:
        _PLAN_CACHE[combo_level] = _build_mm_plan(combo_level)
    return _PLAN_CACHE[combo_level]


def _host_prep(inputs, combo_level=1):
    """Host-side layout prep: x1T, x2rep (fp16, chunk-major) and the scaled
    weight stack (one slice per matmul in plan order)."""
    x1 = np.asarray(inputs["x1"], np.float32)
    x2 = np.asarray(inputs["x2"], np.float32)
    n = x1.shape[0]

    x1t = np.empty((128, 9, n), np.float16)
    for (l1, _) in IN1:
        d1 = 2 * l1 + 1
        blk = x1[:, X1_OFF[l1]:X1_OFF[l1] + MUL * d1].reshape(n, MUL, d1)
        for i in range(d1):
            x1t[:, L1_OFF[l1] + i, :] = blk[:, :, i].astype(np.float16).T

    x2t = x2.astype(np.float16).T  # [9, n]
    x2rep = np.ascontiguousarray(np.broadcast_to(x2t[None, :, :], (128, 9, n)))

    # per-path weight slices (with e3nn path normalization)
    W = {g: np.asarray(inputs[_gname(*g)], np.float32) for g in GROUPS}
    slot = {g: 0 for g in GROUPS}
    path_w = []
    for (l1, p1, l2, p2, l3, p3) in PATHS:
        g = (l3, p3)
        s = slot[g]
        slot[g] += 1
        path_w.append(W[g][s * MUL:(s + 1) * MUL, :] / np.sqrt(np.float32(MULS[g])))

    mm_list, _ = _plan(combo_level)
    n_mm = sum(len(m) for m in mm_list)
    wt = np.empty((MUL, n_mm, MUL), np.float16)  # [u, mm, v]
    pc = 0
    for mms in mm_list:
        for (_, pi, c) in mms:
            wt[:, pc, :] = (path_w[pi] * np.float32(c)).astype(np.float16)
            pc += 1
    assert pc == n_mm
    return x1t, x2rep, wt


def _host_post(y_cores):
    """[cores][128 v, CH, 40 gk, n_chunk] fp16 -> [N, 5120] fp32."""
    y = np.empty((N_TOTAL, 5120), np.float32)
    for ci, yc in enumerate(y_cores):
        # [v, CH, gk, nc] -> [(CH nc), v, gk]
        ycn = yc.astype(np.float32).transpose(1, 3, 0, 2).reshape(NPC, 128, N_GK)
        row0 = ci * NPC
        for g in GROUPS:
            d3 = 2 * g[0] + 1
            base = GK_BASE[g]
            y[row0:row0 + NPC, GOFF[g]:GOFF[g] + MUL * d3] = (
                ycn[:, :, base:base + d3].reshape(NPC, MUL * d3))
    return y


def _xin_core(x1t, x2rep, sl, n_chunk):
    """Fuse per-core x1t/x2rep [128, 9, npc] slices -> [128, CH, 2, 9, nc]."""
    ch = NPC // n_chunk
    out = np.empty((128, ch, 2, 9, n_chunk), np.float16)
    a = x1t[:, :, sl].reshape(128, 9, ch, n_chunk)
    b = x2rep[:, :, sl].reshape(128, 9, ch, n_chunk)
    out[:, :, 0] = a.transpose(0, 2, 1, 3)
    out[:, :, 1] = b.transpose(0, 2, 1, 3)
    return out


# --------------------------------------------------------------- bass build ---

_CACHE = {}


def _build(n_per_core=NPC, n_chunk=256, slot_sz=4, op_bufs=18, yacc_bufs=4,
           stg_bufs=6, pool_op=0, drain_mod=0, combo_level=1, pool_combo=0,
           wt_eng="scalar", stages=15):
    """Build the per-core Bass/Tile program.

    Layouts:
      x1t  [u; i_glob(9), n]   fp16 (host-prepped)
      x2r  [128; j_glob(9), n] fp16 (host-prepped broadcast)
      wt   [u; pass(244), v]   fp16 scaled weights, gk-major pass order
      y    [v; gk(40), n]      fp16 (host untransposes + converts to fp32)

    Per n-chunk: DVE (and optionally Pool) builds outer-product tiles
    OP_(i,*)[u; j, n]; one PSUM-accumulating fp16 matmul per CG nonzero into
    slot_sz-slot accumulator tiles [v; slot, n]; Act/Pool drain each slot
    group to fp16 staging; DMA writes staging straight to y[v; gk, n].
    """
    import concourse.bass as bass
    import concourse.mybir as mybir
    import concourse.tile as tile

    dt = mybir.dt
    NCROWS = n_per_core
    CH = NCROWS // n_chunk

    mm_list, combos = _plan(combo_level)
    n_mm = sum(len(m) for m in mm_list)
    n_gk = len(mm_list)
    assert n_gk % slot_sz == 0
    n_groups = n_gk // slot_sz

    # mm-count per slot group (for the split wt loads)
    grp_np = []
    pc0 = 0
    for t in range(n_groups):
        npg = sum(len(mm_list[t * slot_sz + s]) for s in range(slot_sz))
        grp_np.append((pc0, npg))
        pc0 += npg
    assert pc0 == n_mm

    # slot-groups using each combo (build-order scheduling)
    combo_groups = {}
    for gk, mms in enumerate(mm_list):
        for (operand, _, _) in mms:
            if operand[0] == 'combo':
                combo_groups.setdefault(operand[1], set()).add(gk // slot_sz)

    nc = bass.Bass()

    xin_d = nc.dram_tensor("xin", [128, CH, 2, 9, n_chunk], dt.float16, kind="ExternalInput")
    wt_d = nc.dram_tensor("wt", [MUL, n_mm, MUL], dt.float16, kind="ExternalInput")
    y_d = nc.dram_tensor("y", [128, CH, N_GK, n_chunk], dt.float16, kind="ExternalOutput")

    with tile.TileContext(nc) as tc:
        with (
            tc.tile_pool(name="const", bufs=1) as constp,
            tc.tile_pool(name="op", bufs=op_bufs) as opp,
            tc.tile_pool(name="combo", bufs=2 * len(combos) if combos else 1) as combop,
            tc.tile_pool(name="ystg", bufs=stg_bufs) as ystgp,
            tc.tile_pool(name="yacc", bufs=yacc_bufs, space="PSUM") as yaccp,
        ):
            # input DMAs: one fused x1+x2 load per chunk (SP queue); weights
            # in slices on the Act HWDGE queue (parallel ring), first slice
            # small so matmuls start early
            wt_tiles = [None] * n_groups
            x1c, x2c = [None] * CH, [None] * CH
            wt_q = nc.scalar if wt_eng == "scalar" else nc.sync

            def load_inputs(c):
                # two half-loads on separate HWDGE queues (SP + Act) so the
                # transfers overlap
                t_ = constp.tile([128, 2, 9, n_chunk], dt.float16, name=f"xin{c}")
                nc.sync.dma_start(t_[:, 0], xin_d[:, c, 0])
                nc.sync.dma_start(t_[:, 1], xin_d[:, c, 1])
                x1c[c], x2c[c] = t_, t_

            def load_wt(t):
                p0, npg = grp_np[t]
                wtt = constp.tile([128, npg, 128], dt.float16, name=f"wt_g{t}")
                wt_q.dma_start(wtt[:], wt_d[:, p0:p0 + npg, :])
                wt_tiles[t] = (wtt, p0)

            # combo-free / low-dependency groups first so chunk-0 matmuls
            # never wait on combo builds
            g_perm = [2, 0, 5, 4, 3, 1, 6, 7, 8, 9]
            if n_groups != 10:
                g_perm = list(range(n_groups))
            perm_pos = {t: i for i, t in enumerate(g_perm)}
            load_inputs(0)
            for t in g_perm[:3]:
                load_wt(t)
            for c in range(1, CH):
                load_inputs(c)
                for t in g_perm[3 + (c - 1) * 4:3 + c * 4]:
                    load_wt(t)
            for t in g_perm:
                if wt_tiles[t] is None:
                    load_wt(t)

            dr = 0
            for c in range(CH):
                n0 = c * n_chunk

                op_tiles = {}
                combo_tiles = {}
                if stages & 1:
                    # combo builds interleave with op builds: emit each combo
                    # as soon as its last input i-tile exists
                    by_max_i = {}
                    for q in range(len(combos)):
                        mi = max(ij[0] for ij, _ in combos[q])
                        by_max_i.setdefault(mi, []).append(q)

                    def build_combo(q):
                        pat = combos[q]
                        t_ = combop.tile([128, n_chunk], dt.float16, tag="combo",
                                         name=f"combo_c{c}_q{q}")
                        pm1 = all(abs(abs(r) - 1.0) < 1e-6 for _, r in pat)
                        if pm1:
                            prev = op_tiles[pat[0][0]]
                            for (ijt, rt) in pat[1:]:
                                op = (mybir.AluOpType.add if rt > 0
                                      else mybir.AluOpType.subtract)
                                nc.vector.tensor_tensor(t_[:], prev,
                                                        op_tiles[ijt], op=op)
                                prev = t_[:]
                        else:
                            (ij0, _), (ij1, r1) = pat[0], pat[1]
                            nc.vector.scalar_tensor_tensor(
                                t_[:], op_tiles[ij1], float(r1), op_tiles[ij0],
                                op0=mybir.AluOpType.mult, op1=mybir.AluOpType.add)
                            for (ijt, rt) in pat[2:]:
                                nc.vector.scalar_tensor_tensor(
                                    t_[:], op_tiles[ijt], float(rt), t_[:],
                                    op0=mybir.AluOpType.mult,
                                    op1=mybir.AluOpType.add)
                        combo_tiles[q] = t_[:]

                    for ig in range(9):
                        t_ = opp.tile([128, 9, n_chunk], dt.float16, tag="op",
                                      name=f"op_c{c}_i{ig}")
                        a_b = x1c[c][:, 0, ig, :].unsqueeze(1)
                        a_b = a_b.broadcast_to([128, 9, n_chunk])
                        eng = nc.gpsimd if ig >= 9 - pool_op else nc.vector
                        eng.tensor_mul(t_[:], a_b, x2c[c][:, 1, :, :])
                        for jg in range(9):
                            op_tiles[(ig, jg)] = t_[:, jg, :]
                        for q in sorted(by_max_i.get(ig, []),
                                        key=lambda q: min(
                                            (perm_pos[t] for t in
                                             combo_groups.get(q, ())),
                                            default=99)):
                            build_combo(q)

                for t_idx in g_perm:
                    pc = grp_np[t_idx][0]
                    if not (stages & 2):
                        continue
                    acc = yaccp.tile([128, slot_sz, n_chunk], dt.float32,
                                     tag="yacc", name=f"acc_c{c}_t{t_idx}")
                    ystg = ystgp.tile([128, slot_sz, n_chunk], dt.float16,
                                      tag="ystg", name=f"ystg_c{c}_t{t_idx}")
                    wtt, p0 = wt_tiles[t_idx]
                    # per slot: matmuls sorted by operand availability (op
                    # tile i, combos after their last input i-tile); slots
                    # ordered earliest-finishable first. Accumulation runs
                    # stay contiguous per slot (interleaving them corrupts
                    # PSUM accumulation).
                    slot_entries = []
                    pcx = p0
                    for s in range(slot_sz):
                        mms = mm_list[t_idx * slot_sz + s]
                        ent = []
                        for (operand, _, _) in mms:
                            if operand[0] == 'op':
                                rank = 2 * operand[1][0]
                            else:
                                rank = 2 * max(ij[0] for ij, _ in
                                               combos[operand[1]]) + 1
                            ent.append((rank, pcx, operand))
                            pcx += 1
                        ent.sort(key=lambda e: e[0])
                        slot_entries.append((max(e[0] for e in ent), s, ent))
                    slot_entries.sort(key=lambda x: (x[0], x[1]))
                    for (_, s, ent) in slot_entries:
                        nmm = len(ent)
                        for m, (rank, pidx, operand) in enumerate(ent):
                            if operand[0] == 'op':
                                rhs = op_tiles[operand[1]]
                            else:
                                rhs = combo_tiles[operand[1]]
                            nc.tensor.matmul(
                                acc[:, s, :],
                                wtt[:, pidx - p0, :],
                                rhs if stages & 1 else wtt[:, pidx - p0, :],
                                start=(m == 0),
                                stop=(m == nmm - 1),
                            )
                    # drain PSUM -> fp16 staging (Act mostly, DVE every drain_mod-th;
                    # gpsimd cannot read PSUM)
                    if drain_mod and dr % drain_mod == drain_mod - 1:
                        nc.vector.tensor_copy(ystg[:], acc[:])
                    else:
                        nc.scalar.copy(ystg[:], acc[:])
                    dr += 1
                    if stages & 8:
                        gk0 = t_idx * slot_sz
                        nc.sync.dma_start(
                            y_d[:, c, gk0:gk0 + slot_sz, :], ystg[:])

    return nc


def _hoist_waits(nc, max_waits=1):
    """Walrus in this toolchain rejects instructions with more than one
    sync-wait command; hoist extras onto single-wait NOP/Drain carriers that
    precede the instruction on the same engine."""
    import concourse.mybir as mybir

    n_hoisted = 0
    for bb in nc.main_func.blocks:
        new_list = []
        for ins in bb.instructions:
            si = ins.sync_info
            if si is not None and si.on_wait and len(si.on_wait) > max_waits:
                waits = list(si.on_wait)
                keep, hoist = waits[:max_waits], waits[max_waits:]
                for w in hoist:
                    # NoOp (not Drain): Drain flushes the engine pipeline
                    # (~2.2us on a busy PE); NoOp just carries the wait.
                    carrier = mybir.InstNoOp(
                        name=nc.get_next_instruction_name(),
                        bass_nofuse=True,
                        engine=ins.engine,
                        sync_info=mybir.SyncInfo(on_wait=[w], on_update=[]),
                    )
                    new_list.append(carrier)
                    n_hoisted += 1
                ins.sync_info = mybir.SyncInfo(on_wait=keep, on_update=list(si.on_update))
            new_list.append(ins)
        bb.instructions[:] = new_list
    return n_hoisted


def _get_program(**kw):
    key = tuple(sorted(kw.items()))
    if key not in _CACHE:
        nc = _build(**kw)
        _hoist_waits(nc)
        _CACHE[key] = nc
    return _CACHE[key]


def _run(inputs, trace=False, **build_kw):
    from concourse import bass_utils

    nc = _get_program(**build_kw)
    n_chunk = build_kw.get("n_chunk", 256)
    x1t, x2rep, wt = _host_prep(inputs, build_kw.get("combo_level", 1))

    in_maps = []
    for core in range(N_CORES):
        sl = slice(core * NPC, (core + 1) * NPC)
        in_maps.append({
            "xin": _xin_core(x1t, x2rep, sl, n_chunk),
            "wt": wt,
        })

    res = bass_utils.run_bass_kernel_spmd(
        nc, in_maps, core_ids=list(range(N_CORES)), trace=trace,
    )
    y = _host_post([r["y"] for r in res.results])
    return y, res


def kernel(**inputs):
    y, _ = _run(inputs)
    return y


def _make_sharded_fn(nc):
    """Mimic bass2jax.run_bass_via_pjrt's multi-core path, returning
    (sharded_fn, in_names, out_names, out_avals, mesh, n_params)."""
    import jax
    from jax.sharding import Mesh, PartitionSpec
    from jax.experimental.shard_map import shard_map
    from concourse import bass2jax, mybir
    import numpy as _np

    bass2jax.install_neuronx_cc_hook()
    partition_name = nc.partition_id_tensor.name if nc.partition_id_tensor else None
    in_names, out_names, out_avals = [], [], []
    for alloc in nc.m.functions[0].allocations:
        if not isinstance(alloc, mybir.MemoryLocationSet):
            continue
        name = alloc.memorylocations[0].name
        if alloc.kind == "ExternalInput":
            if name != partition_name:
                in_names.append(name)
        elif alloc.kind == "ExternalOutput":
            out_names.append(name)
            out_avals.append(jax.core.ShapedArray(tuple(alloc.tensor_shape), mybir.dt.np(alloc.dtype)))
    n_params = len(in_names)
    all_in_names = list(in_names) + list(out_names)
    if partition_name is not None:
        all_in_names.append(partition_name)
    donate = tuple(range(n_params, n_params + len(out_avals)))

    def _body(*args):
        operands = list(args)
        if partition_name is not None:
            operands.append(bass2jax.partition_id_tensor())
        outs = bass2jax._bass_exec_p.bind(
            *operands,
            out_avals=tuple(out_avals),
            in_names=tuple(all_in_names),
            out_names=tuple(out_names),
            lowering_input_output_aliases=(),
            sim_require_finite=True,
            sim_require_nnan=True,
            nc=nc,
        )
        return tuple(outs)

    devices = jax.devices()[:N_CORES]
    mesh = Mesh(_np.asarray(devices), ("core",))
    in_specs = (PartitionSpec("core"),) * (n_params + len(out_avals))
    out_specs = (PartitionSpec("core"),) * len(out_names)
    sharded = jax.jit(
        shard_map(_body, mesh=mesh, in_specs=in_specs, out_specs=out_specs, check_rep=False),
        donate_argnums=donate,
        keep_unused=True,
    )
    return sharded, in_names, out_names, out_avals, mesh, n_params


def bench(inputs, iters=30, warmup=3, **build_kw):
    """Time repeated on-device executions (inputs device-resident).

    Returns (per_iter_ns, y) where per_iter_ns is the average over the
    timed iterations.
    """
    import time as _time
    import jax
    from jax.sharding import NamedSharding, PartitionSpec

    nc = _get_program(**build_kw)
    n_chunk = build_kw.get("n_chunk", 256)
    x1t, x2rep, wt = _host_prep(inputs, build_kw.get("combo_level", 1))
    per_core = {
        "xin": [_xin_core(x1t, x2rep, slice(c * NPC, (c + 1) * NPC), n_chunk) for c in range(N_CORES)],
        "wt": [wt] * N_CORES,
    }
    sharded, in_names, out_names, out_avals, mesh, n_params = _make_sharded_fn(nc)
    sh = NamedSharding(mesh, PartitionSpec("core"))
    dev_in = [
        jax.device_put(np.concatenate(per_core[name], axis=0), sh) for name in in_names
    ]

    def zeros():
        return [
            jax.device_put(np.zeros((N_CORES * a.shape[0], *a.shape[1:]), a.dtype), sh)
            for a in out_avals
        ]

    outs = None
    for _ in range(warmup):
        outs = sharded(*dev_in, *zeros())
        jax.block_until_ready(outs)

    # pre-stage zero buffers outside the timed region
    zs = [zeros() for _ in range(iters)]
    jax.block_until_ready(zs)
    t0 = _time.perf_counter()
    res = [sharded(*dev_in, *z) for z in zs]
    jax.block_until_ready(res)
    dt = (_time.perf_counter() - t0) / iters

    y_cat = np.asarray(res[-1][out_names.index("y")])
    y = _host_post([y_cat[c * 128:(c + 1) * 128] for c in range(N_CORES)])
    return dt * 1e9, y


if __name__ == "__main__":
    print("passes:", N_PASSES, "used_ij:", len(USED_IJ), "gk:", N_GK)
